# revision 14
# baseline (speedup 1.0000x reference)
"""Trainium2 Bass kernel for nn_AttnLayerV3 (differential attention layer).

Tensor-parallel over heads across 8 NeuronCores:
  - core c owns q-heads {2c, 2c+1} and kv-head c//2 (duplicated per core pair)
  - per-core: Q/K/V projections, RoPE, two-stream causal softmax attention
    (computed transposed: scores (k, q) so attention weights are directly the
    lhsT of the AV matmul), per-head GroupNorm, partial output projection
  - host: shards/permutes weights, gathers the 8 partial outputs and sums.

All matmuls run in bf16 with fp32 PSUM accumulation (verified ~7e-3 rel err
vs the fp32 reference).
"""

import numpy as np
import ml_dtypes

import concourse.bass as bass
import concourse.bacc as bacc
import concourse.tile as tile
import concourse.mybir as mybir
from concourse.bass_utils import run_bass_kernel_spmd
from concourse.masks import make_identity, make_upper_triangular

bf16 = ml_dtypes.bfloat16

B, T, D = 2, 1024, 2048
H, KV, DH = 16, 4, 128
NCORES = 8
HPC = H // NCORES          # q heads per core = 2
TOK = B * T                # 2048
LAMBDA_INIT = 0.8 - 0.6 * float(np.exp(-0.3 * 1))
GN_EPS = 1e-5
ROPE_BASE = 10000.0

KB = 16                    # contraction blocks of 128 over D
TSB = TOK // 512           # 4 token superblocks of 512
QB = T // 128              # 8 q/k blocks of 128 per batch
NTB = TOK // 128           # 16 token blocks of 128


def build_program(lam: float):
    f32 = mybir.dt.float32
    bf = mybir.dt.bfloat16
    nc = bacc.Bacc("TRN2", target_bir_lowering=False, debug=False,
                   num_devices=NCORES)

    xT_d = nc.dram_tensor("xT", (128, KB, TOK), bf, kind="ExternalInput").ap()
    wq_d = nc.dram_tensor("wqT", (4, 128, KB, 128), bf, kind="ExternalInput").ap()
    wk_d = nc.dram_tensor("wkT", (2, 128, KB, 128), bf, kind="ExternalInput").ap()
    wv_d = nc.dram_tensor("wvT", (128, KB, 256), bf, kind="ExternalInput").ap()
    wo_d = nc.dram_tensor("woT", (128, 4, D), bf, kind="ExternalInput").ap()
    tbl_d = nc.dram_tensor("tbl", (128, TOK), bf, kind="ExternalInput").ap()
    tb2_d = nc.dram_tensor("tbl2", (128, TOK), bf, kind="ExternalInput").ap()
    out_d = nc.dram_tensor("out", (TOK, D), bf, kind="ExternalOutput").ap()

    with tile.TileContext(nc) as tc:
        _body(tc, xT_d, wq_d, wk_d, wv_d, wo_d, tbl_d, tb2_d, out_d, lam)
    nc.compile()
    return nc


def _body(tc, xT_d, wq_d, wk_d, wv_d, wo_d, tbl_d, tb2_d, out_d, lam):
    nc = tc.nc
    f32 = mybir.dt.float32
    bf = mybir.dt.bfloat16
    mult = mybir.AluOpType.mult
    sub = mybir.AluOpType.subtract
    add = mybir.AluOpType.add

    import contextlib
    ctx = contextlib.ExitStack()
    with ctx:
        # ---- pools -------------------------------------------------------
        big = ctx.enter_context(tc.tile_pool(name="big", bufs=1))
        wpool = ctx.enter_context(tc.tile_pool(name="wq", bufs=2))
        tmp = ctx.enter_context(tc.tile_pool(name="tmp", bufs=3))
        rtmp = ctx.enter_context(tc.tile_pool(name="rtmp", bufs=3))
        opool = ctx.enter_context(tc.tile_pool(name="o", bufs=3))
        spool = ctx.enter_context(tc.tile_pool(name="s", bufs=4))
        epool = ctx.enter_context(tc.tile_pool(name="e", bufs=2))
        stage = ctx.enter_context(tc.tile_pool(name="stage", bufs=4))
        pmm = ctx.enter_context(tc.tile_pool(name="pmm", bufs=4, space="PSUM"))
        pav = ctx.enter_context(tc.tile_pool(name="pav", bufs=2, space="PSUM"))

        # ---- constants ---------------------------------------------------
        ident = big.tile([128, 128], bf, tag="ident")
        make_identity(nc, ident[:])
        tri = big.tile([128, 128], bf, tag="tri")
        make_upper_triangular(nc, tri[:], val=1.0, diag=True)
        tbl = big.tile([128, TOK], bf, tag="tbl")        # cos rows 0:64, sin 64:128
        nc.sync.dma_start(tbl[:], tbl_d)
        tbl2 = big.tile([128, TOK], bf, tag="tbl2")      # sin rows 0:64, cos 64:128
        nc.sync.dma_start(tbl2[:], tb2_d)
        cosv = tbl[0:64, :]      # base 0
        sinv = tbl[64:128, :]    # base 64
        sinv0 = tbl2[0:64, :]    # sin at base 0
        cosv64 = tbl2[64:128, :]  # cos at base 64

        # ---- resident tensors -------------------------------------------
        xsb = big.tile([128, KB, TOK], bf, tag="xsb")
        for ts in range(TSB):
            for k in range(KB):
                nc.sync.dma_start(xsb[:, k, ts * 512:(ts + 1) * 512],
                                  xT_d[:, k, ts * 512:(ts + 1) * 512])

        # q streams (4 blocks: h0s0 h0s1 h1s0 h1s1) then k streams (2 blocks)
        qk = big.tile([128, 6, TOK], bf, tag="qk")
        vsb = big.tile([128, NTB, 257], bf, tag="vsb")
        nc.vector.memset(vsb[:, :, 256:257], 1.0)
        osbT = big.tile([128, 4, TOK], bf, tag="osbT")   # o transposed, ch-major
        wo_sb = big.tile([128, 4, D], bf, tag="wo")

        # ---- phase 1: Q/K projections + RoPE ----------------------------
        # emit k streams first so attention can start early
        cb_order = [(4, 0), (5, 1), (0, 0), (1, 1), (2, 2), (3, 3)]
        for qkcb, wcol in cb_order:
            is_k = qkcb >= 4
            wt = wpool.tile([128, KB, 128], bf, tag="w")
            src = wk_d if is_k else wq_d
            nc.sync.dma_start(wt[:], src[wcol])
            for ts in range(TSB):
                ps = pmm.tile([128, 512], f32, tag="mm")
                for k in range(KB):
                    nc.tensor.matmul(ps[:], wt[:, k], xsb[:, k, ts * 512:(ts + 1) * 512],
                                     start=(k == 0), stop=(k == KB - 1))
                # RoPE: psum -> bf16 sbuf -> rotate into qk
                tsl = slice(ts * 512, (ts + 1) * 512)
                raw = tmp.tile([128, 512], bf, tag="raw")
                nc.scalar.copy(raw[:], ps[:])
                x1, x2 = raw[0:64, :], raw[64:128, :]
                cs, sn = cosv[:, tsl], sinv[:, tsl]
                t1 = rtmp.tile([64, 512], bf, tag="t1")
                t2 = rtmp.tile([64, 512], bf, tag="t2")
                nc.vector.tensor_tensor(t1[:], x2, sn, mult)
                nc.vector.tensor_tensor(t2[:], x1, cs, mult)
                nc.vector.tensor_tensor(qk[0:64, qkcb, tsl], t2[:], t1[:], sub)
                t3 = rtmp.tile([64, 512], bf, tag="t1")
                t4 = rtmp.tile([64, 512], bf, tag="t2")
                nc.vector.tensor_tensor(t3[:], x2, cosv64[:, tsl], mult)
                nc.vector.tensor_tensor(t4[:], x1, sinv0[:, tsl], mult)
                nc.vector.tensor_tensor(qk[64:128, qkcb, tsl], t4[:], t3[:], add)

        # ---- phase 1b: V projection -------------------------------------
        wv = big.tile([128, KB, 256], bf, tag="wv")
        for k4 in range(4):
            nc.sync.dma_start(wv[:, k4 * 4:(k4 + 1) * 4], wv_d[:, k4 * 4:(k4 + 1) * 4])
        for cb in range(4):
            nc.sync.dma_start(wo_sb[:, cb], wo_d[:, cb])
        for tb in range(NTB):
            ps = pmm.tile([128, 512], f32, tag="mm")
            for k in range(KB):
                nc.tensor.matmul(ps[:, 0:256], xsb[:, k, tb * 128:(tb + 1) * 128],
                                 wv[:, k], start=(k == 0), stop=(k == KB - 1))
            nc.scalar.copy(vsb[:, tb, 0:256], ps[:, 0:256])

        # ---- phase 2: attention per (head, batch) -----------------------
        for h in range(HPC):
            for b in range(B):
                for J in range(2):          # q superblocks of 512 per batch
                    nk = 4 * (J + 1)
                    qsl = slice(b * T + J * 512, b * T + (J + 1) * 512)
                    e1 = epool.tile([128, QB, 512], bf, tag="e1")
                    e2 = epool.tile([128, QB, 512], bf, tag="e2")
                    for s, et in ((0, e1), (1, e2)):
                        qblk = qk[:, 2 * h + s, qsl]
                        for i in range(nk):
                            ksl = slice(b * T + i * 128, b * T + (i + 1) * 128)
                            ps = pmm.tile([128, 512], f32, tag="mm")
                            nc.tensor.matmul(ps[:], qk[:, 4 + s, ksl], qblk,
                                             start=True, stop=True)
                            nc.scalar.activation(et[:, i, :], ps[:],
                                                 mybir.ActivationFunctionType.Exp)
                            for jl in range(4):
                                jg = 4 * J + jl
                                qq = slice(jl * 128, (jl + 1) * 128)
                                if i > jg:
                                    nc.gpsimd.memset(et[:, i, qq], 0.0)
                                elif i == jg:
                                    nc.vector.tensor_tensor(et[:, i, qq],
                                                            et[:, i, qq], tri[:], mult)
                    for jl in range(4):
                        jg = 4 * J + jl
                        qq = slice(jl * 128, (jl + 1) * 128)
                        u1f = pav.tile([128, 512], f32, tag="u1", name="u1")
                        u2f = pav.tile([128, 512], f32, tag="u2", name="u2")
                        u1 = u1f[:, 0:257]
                        u2 = u2f[:, 0:257]
                        for i in range(jg + 1):
                            vt = vsb[:, b * QB + i, :]
                            nc.tensor.matmul(u1, e1[:, i, qq], vt,
                                             start=(i == 0), stop=(i == jg),
                                             skip_group_check=True)
                            nc.tensor.matmul(u2, e2[:, i, qq], vt,
                                             start=(i == 0), stop=(i == jg),
                                             skip_group_check=True)
                        r1 = spool.tile([128, 1], f32, tag="r1")
                        r2 = spool.tile([128, 1], f32, tag="r2")
                        nc.vector.reciprocal(r1[:], u1[:, 256:257])
                        nc.vector.reciprocal(r2[:], u2[:, 256:257])
                        nc.vector.tensor_scalar_mul(r2[:], r2[:], -lam)
                        o1 = opool.tile([128, 256], bf, tag="o1")
                        nc.vector.tensor_scalar_mul(o1[:], u1[:, 0:256], r1[:])
                        nc.vector.scalar_tensor_tensor(o1[:], u2[:, 0:256], r2[:],
                                                       o1[:], mult, add)
                        # GroupNorm over the head's 256 channels
                        st6 = spool.tile([128, 6], f32, tag="st6")
                        mv = spool.tile([128, 2], f32, tag="mv")
                        nc.vector.bn_stats(st6[:], o1[:])
                        nc.vector.bn_aggr(mv[:], st6[:])
                        sd = spool.tile([128, 1], f32, tag="sd")
                        rstd = spool.tile([128, 1], f32, tag="rstd")
                        nc.vector.tensor_scalar_add(sd[:], mv[:, 1:2], GN_EPS)
                        nc.scalar.sqrt(sd[:], sd[:])
                        nc.vector.reciprocal(rstd[:], sd[:])
                        o2 = opool.tile([128, 256], bf, tag="o2")
                        nc.vector.tensor_scalar(o2[:], o1[:], mv[:, 0:1], rstd[:],
                                                sub, mult)
                        # transpose (q,ch)->(ch,q) into resident osbT
                        for half in range(2):
                            pst = pmm.tile([128, 512], bf, tag="mm", name="pst")
                            nc.tensor.transpose(pst[:, 0:128],
                                                o2[:, half * 128:(half + 1) * 128],
                                                ident[:])
                            nc.vector.tensor_copy(
                                osbT[:, 2 * h + half, b * T + jg * 128:b * T + (jg + 1) * 128],
                                pst[:, 0:128])

        # ---- phase 3: partial output projection -------------------------
        for tb in range(NTB):
            pso = [pmm.tile([128, 512], f32, tag="mm", name=f"pso{n}")
                   for n in range(4)]
            for cb in range(4):
                for n in range(4):
                    nc.tensor.matmul(pso[n][:], osbT[:, cb, tb * 128:(tb + 1) * 128],
                                     wo_sb[:, cb, n * 512:(n + 1) * 512],
                                     start=(cb == 0), stop=(cb == 3),
                                     skip_group_check=True)
            for n in range(4):
                so = stage.tile([128, 512], bf, tag="so")
                nc.scalar.copy(so[:], pso[n][:])
                nc.sync.dma_start(out_d[tb * 128:(tb + 1) * 128,
                                        n * 512:(n + 1) * 512], so[:])


# ------------------------- host side ------------------------------------

_ROPE_PERM = np.concatenate([np.arange(0, DH, 2), np.arange(1, DH, 2)])


def _prep(x, Wq, Wk, Wv, Wo, lambda_q1, lambda_k1, lambda_q2, lambda_k2,
          gn_weight, gn_bias, pos):
    lam = float(np.exp(np.sum(lambda_q1 * lambda_k1))
                - np.exp(np.sum(lambda_q2 * lambda_k2)) + LAMBDA_INIT)
    scale = DH ** -0.5

    posf = pos.astype(np.float64)
    inv = 1.0 / (ROPE_BASE ** (np.arange(0, DH, 2, dtype=np.float32) / DH))
    freqs = (posf[:, None] * inv[None, :].astype(np.float64)).astype(np.float32)
    cosv = np.cos(freqs).T          # (64, T)
    sinv = np.sin(freqs).T
    tbl = np.concatenate([np.tile(cosv, (1, B)), np.tile(sinv, (1, B))],
                         axis=0).astype(bf16)          # (128, TOK) [cos;sin]
    tbl = np.ascontiguousarray(tbl)
    tbl2 = np.concatenate([np.tile(sinv, (1, B)), np.tile(cosv, (1, B))],
                          axis=0).astype(bf16)         # (128, TOK) [sin;cos]
    tbl2 = np.ascontiguousarray(tbl2)

    # x transposed: (D, B*T) -> (128, KB, TOK)
    xT = x.reshape(TOK, D).T.astype(np.float32)
    xT3 = np.ascontiguousarray(
        xT.reshape(KB, 128, TOK).transpose(1, 0, 2)).astype(bf16)

    # Wq: (H,2,DH,D), rope-permute DH, fold score scale
    Wq4 = (Wq.reshape(H, 2, DH, D)[:, :, _ROPE_PERM, :] * scale).astype(np.float32)
    Wk4 = Wk.reshape(KV, 2, DH, D)[:, :, _ROPE_PERM, :].astype(np.float32)
    Wv3 = Wv.reshape(KV, 2 * DH, D).astype(np.float32)

    s1 = 1.0 - LAMBDA_INIT
    Wo_f = (Wo * (gn_weight * s1)[None, :]).astype(np.float32)   # (D, 4096)
    bias_out = (gn_bias * s1).astype(np.float32) @ Wo.T.astype(np.float32)

    def to_sb(w2d, cols):           # (D, cols) -> (128, KB, cols) bf16
        return np.ascontiguousarray(
            w2d.reshape(KB, 128, cols).transpose(1, 0, 2)).astype(bf16)

    in_maps = []
    for c in range(NCORES):
        wqT = Wq4[2 * c:2 * c + 2].reshape(512, D).T          # (D, 512)
        wkT = Wk4[c // 2].reshape(256, D).T                   # (D, 256)
        wvT = Wv3[c // 2].T                                   # (D, 256)
        woT = Wo_f[:, 512 * c:512 * c + 512].T                # (512 ch, D out)
        wo3 = np.ascontiguousarray(
            woT.reshape(4, 128, D).transpose(1, 0, 2)).astype(bf16)
        wq_stack = np.stack([to_sb(wqT[:, cb * 128:(cb + 1) * 128], 128)
                             for cb in range(4)])             # (4,128,KB,128)
        wk_stack = np.stack([to_sb(wkT[:, cb * 128:(cb + 1) * 128], 128)
                             for cb in range(2)])             # (2,128,KB,128)
        in_maps.append({
            "xT": xT3,
            "wqT": wq_stack,
            "wkT": wk_stack,
            "wvT": to_sb(wvT, 256),
            "woT": wo3,
            "tbl": tbl,
            "tbl2": tbl2,
        })
    return lam, in_maps, bias_out


LAST_RESULT = None


def kernel(**inputs):
    global LAST_RESULT
    inputs = {k: np.asarray(v) for k, v in inputs.items()}
    lam, in_maps, bias_out = _prep(**inputs)
    nc = build_program(lam)
    res = run_bass_kernel_spmd(nc, in_maps, core_ids=list(range(NCORES)))
    LAST_RESULT = res
    out = np.zeros((TOK, D), np.float32)
    for c in range(NCORES):
        out += res.results[c]["out"].astype(np.float32)
    out += bias_out[None, :]
    return out.reshape(B, T, D).astype(np.float32)


if __name__ == "__main__":
    import reference
    ins = {k: np.asarray(v) for k, v in reference.setup_inputs().items()}
    got = kernel(**ins)
    exp = np.asarray(reference.reference(**ins))
    rel = np.linalg.norm(got - exp) / np.linalg.norm(exp)
    print("rel err:", rel)


# revision 16
# speedup vs baseline: 1.0805x; 1.0805x over previous
"""Trainium2 Bass kernel for nn_AttnLayerV3 (differential attention layer).

Tensor-parallel over heads across 8 NeuronCores:
  - core c owns q-heads {2c, 2c+1} and kv-head c//2 (duplicated per core pair)
  - per-core: Q/K/V projections, RoPE, two-stream causal softmax attention
    (computed transposed: scores (k, q) so attention weights are directly the
    lhsT of the AV matmul), per-head GroupNorm, partial output projection
  - host: shards/permutes weights, gathers the 8 partial outputs and sums.

All matmuls run in bf16 with fp32 PSUM accumulation (verified ~7e-3 rel err
vs the fp32 reference).
"""

import numpy as np
import ml_dtypes

import concourse.bass as bass
import concourse.bacc as bacc
import concourse.tile as tile
import concourse.mybir as mybir
from concourse.bass_utils import run_bass_kernel_spmd
from concourse.masks import make_identity, make_upper_triangular

bf16 = ml_dtypes.bfloat16

B, T, D = 2, 1024, 2048
H, KV, DH = 16, 4, 128
NCORES = 8
HPC = H // NCORES          # q heads per core = 2
TOK = B * T                # 2048
LAMBDA_INIT = 0.8 - 0.6 * float(np.exp(-0.3 * 1))
GN_EPS = 1e-5
ROPE_BASE = 10000.0

KB = 16                    # contraction blocks of 128 over D
TSB = TOK // 512           # 4 token superblocks of 512
QB = T // 128              # 8 q/k blocks of 128 per batch
NTB = TOK // 128           # 16 token blocks of 128


def build_program(lam: float):
    f32 = mybir.dt.float32
    bf = mybir.dt.bfloat16
    nc = bacc.Bacc("TRN2", target_bir_lowering=False, debug=False,
                   num_devices=NCORES)

    xT_d = nc.dram_tensor("xT", (TSB, 128, KB, 512), bf, kind="ExternalInput").ap()
    wq_d = nc.dram_tensor("wqT", (4, 128, KB, 128), bf, kind="ExternalInput").ap()
    wk_d = nc.dram_tensor("wkT", (2, 128, KB, 128), bf, kind="ExternalInput").ap()
    wv_d = nc.dram_tensor("wvT", (128, KB, 256), bf, kind="ExternalInput").ap()
    wo_d = nc.dram_tensor("woT", (128, 4, D), bf, kind="ExternalInput").ap()
    tbl_d = nc.dram_tensor("tbl", (128, TOK), bf, kind="ExternalInput").ap()
    tb2_d = nc.dram_tensor("tbl2", (128, TOK), bf, kind="ExternalInput").ap()
    out_d = nc.dram_tensor("out", (TOK, D), bf, kind="ExternalOutput").ap()

    with tile.TileContext(nc) as tc:
        _body(tc, xT_d, wq_d, wk_d, wv_d, wo_d, tbl_d, tb2_d, out_d, lam)
    nc.compile()
    return nc


def _body(tc, xT_d, wq_d, wk_d, wv_d, wo_d, tbl_d, tb2_d, out_d, lam):
    nc = tc.nc
    f32 = mybir.dt.float32
    bf = mybir.dt.bfloat16
    mult = mybir.AluOpType.mult
    sub = mybir.AluOpType.subtract
    add = mybir.AluOpType.add

    import contextlib
    ctx = contextlib.ExitStack()
    with ctx:
        # ---- pools -------------------------------------------------------
        big = ctx.enter_context(tc.tile_pool(name="big", bufs=1))
        wpool = ctx.enter_context(tc.tile_pool(name="wq", bufs=2))
        tmp = ctx.enter_context(tc.tile_pool(name="tmp", bufs=3))
        rtmp = ctx.enter_context(tc.tile_pool(name="rtmp", bufs=3))
        opool = ctx.enter_context(tc.tile_pool(name="o", bufs=3))
        spool = ctx.enter_context(tc.tile_pool(name="s", bufs=4))
        epool = ctx.enter_context(tc.tile_pool(name="e", bufs=2))
        stage = ctx.enter_context(tc.tile_pool(name="stage", bufs=4))
        pmm = ctx.enter_context(tc.tile_pool(name="pmm", bufs=4, space="PSUM"))
        pav = ctx.enter_context(tc.tile_pool(name="pav", bufs=2, space="PSUM"))

        # ---- constants ---------------------------------------------------
        ident = big.tile([128, 128], bf, tag="ident")
        make_identity(nc, ident[:])
        tri = big.tile([128, 128], bf, tag="tri")
        make_upper_triangular(nc, tri[:], val=1.0, diag=True)

        # weights for the two K streams first (unblock first matmuls)
        wts = {}
        for cb in range(2):
            wt = wpool.tile([128, KB, 128], bf, tag="w", name=f"wtk{cb}")
            nc.sync.dma_start(wt[:], wk_d[cb])
            wts[4 + cb] = wt

        tbl = big.tile([128, TOK], bf, tag="tbl")        # cos rows 0:64, sin 64:128
        nc.sync.dma_start(tbl[:], tbl_d)
        tbl2 = big.tile([128, TOK], bf, tag="tbl2")      # sin rows 0:64, cos 64:128
        nc.sync.dma_start(tbl2[:], tb2_d)
        cosv = tbl[0:64, :]      # base 0
        sinv = tbl[64:128, :]    # base 64
        sinv0 = tbl2[0:64, :]    # sin at base 0
        cosv64 = tbl2[64:128, :]  # cos at base 64

        # ---- resident tensors -------------------------------------------
        xsb = big.tile([128, KB, TOK], bf, tag="xsb")
        nc.sync.dma_start(xsb[:, :, 0:512], xT_d[0])
        for cb in range(4):
            wt = wpool.tile([128, KB, 128], bf, tag="w", name=f"wtq{cb}")
            nc.sync.dma_start(wt[:], wq_d[cb])
            wts[cb] = wt
        for ts in range(1, TSB):
            nc.sync.dma_start(xsb[:, :, ts * 512:(ts + 1) * 512], xT_d[ts])

        # q streams (4 blocks: h0s0 h0s1 h1s0 h1s1) then k streams (2 blocks)
        qk = big.tile([128, 6, TOK], bf, tag="qk")
        vsb = big.tile([128, NTB, 257], bf, tag="vsb")
        nc.vector.memset(vsb[:, :, 256:257], 1.0)
        osbT = big.tile([128, 4, TOK], bf, tag="osbT")   # o transposed, ch-major
        wo_sb = big.tile([128, 4, D], bf, tag="wo")

        # ---- phase 1: Q/K projections + RoPE ----------------------------
        # emit k streams first so attention can start early
        cb_order = [(4, 0), (5, 1), (0, 0), (1, 1), (2, 2), (3, 3)]
        for qkcb, wcol in cb_order:
            wt = wts[qkcb]
            for ts in range(TSB):
                ps = pmm.tile([128, 512], f32, tag="mm")
                for k in range(KB):
                    nc.tensor.matmul(ps[:], wt[:, k], xsb[:, k, ts * 512:(ts + 1) * 512],
                                     start=(k == 0), stop=(k == KB - 1))
                # RoPE: psum -> bf16 sbuf -> rotate into qk
                tsl = slice(ts * 512, (ts + 1) * 512)
                raw = tmp.tile([128, 512], bf, tag="raw")
                nc.scalar.copy(raw[:], ps[:])
                x1, x2 = raw[0:64, :], raw[64:128, :]
                cs, sn = cosv[:, tsl], sinv[:, tsl]
                t1 = rtmp.tile([64, 512], bf, tag="t1")
                t2 = rtmp.tile([64, 512], bf, tag="t2")
                nc.vector.tensor_tensor(t1[:], x2, sn, mult)
                nc.vector.tensor_tensor(t2[:], x1, cs, mult)
                nc.vector.tensor_tensor(qk[0:64, qkcb, tsl], t2[:], t1[:], sub)
                t3 = rtmp.tile([64, 512], bf, tag="t1")
                t4 = rtmp.tile([64, 512], bf, tag="t2")
                nc.vector.tensor_tensor(t3[:], x2, cosv64[:, tsl], mult)
                nc.vector.tensor_tensor(t4[:], x1, sinv0[:, tsl], mult)
                nc.vector.tensor_tensor(qk[64:128, qkcb, tsl], t4[:], t3[:], add)

        # ---- phase 1b: V projection -------------------------------------
        wv = big.tile([128, KB, 256], bf, tag="wv")
        for k4 in range(4):
            nc.sync.dma_start(wv[:, k4 * 4:(k4 + 1) * 4], wv_d[:, k4 * 4:(k4 + 1) * 4])
        for cb in range(4):
            nc.sync.dma_start(wo_sb[:, cb], wo_d[:, cb])
        for tb in range(NTB):
            ps = pmm.tile([128, 512], f32, tag="mm")
            for k in range(KB):
                nc.tensor.matmul(ps[:, 0:256], xsb[:, k, tb * 128:(tb + 1) * 128],
                                 wv[:, k], start=(k == 0), stop=(k == KB - 1))
            nc.scalar.copy(vsb[:, tb, 0:256], ps[:, 0:256])

        # ---- phase 2: attention per (head, batch) -----------------------
        for h in range(HPC):
            for b in range(B):
                for J in range(2):          # q superblocks of 512 per batch
                    nk = 4 * (J + 1)
                    qsl = slice(b * T + J * 512, b * T + (J + 1) * 512)
                    e1 = epool.tile([128, QB, 512], bf, tag="e1")
                    e2 = epool.tile([128, QB, 512], bf, tag="e2")
                    for s, et in ((0, e1), (1, e2)):
                        qblk = qk[:, 2 * h + s, qsl]
                        for i in range(nk):
                            ksl = slice(b * T + i * 128, b * T + (i + 1) * 128)
                            ps = pmm.tile([128, 512], f32, tag="mm")
                            nc.tensor.matmul(ps[:], qk[:, 4 + s, ksl], qblk,
                                             start=True, stop=True)
                            nc.scalar.activation(et[:, i, :], ps[:],
                                                 mybir.ActivationFunctionType.Exp)
                            if i >= 4 * J:      # diagonal sub-block masking
                                jl = i - 4 * J
                                qq = slice(jl * 128, (jl + 1) * 128)
                                nc.vector.tensor_tensor(et[:, i, qq],
                                                        et[:, i, qq], tri[:], mult)
                    for jl in range(4):
                        jg = 4 * J + jl
                        qq = slice(jl * 128, (jl + 1) * 128)
                        u1f = pav.tile([128, 512], f32, tag="u1", name="u1")
                        u2f = pav.tile([128, 512], f32, tag="u2", name="u2")
                        u1 = u1f[:, 0:257]
                        u2 = u2f[:, 0:257]
                        for i in range(jg + 1):
                            vt = vsb[:, b * QB + i, :]
                            nc.tensor.matmul(u1, e1[:, i, qq], vt,
                                             start=(i == 0), stop=(i == jg),
                                             skip_group_check=True)
                            nc.tensor.matmul(u2, e2[:, i, qq], vt,
                                             start=(i == 0), stop=(i == jg),
                                             skip_group_check=True)
                        r1 = spool.tile([128, 1], f32, tag="r1")
                        r2 = spool.tile([128, 1], f32, tag="r2")
                        nc.vector.reciprocal(r1[:], u1[:, 256:257])
                        nc.vector.reciprocal(r2[:], u2[:, 256:257])
                        nc.vector.tensor_scalar_mul(r2[:], r2[:], -lam)
                        u1b = opool.tile([128, 256], bf, tag="u1b")
                        u2b = opool.tile([128, 256], bf, tag="u2b")
                        nc.vector.tensor_copy(u1b[:], u1[:, 0:256])
                        nc.vector.tensor_copy(u2b[:], u2[:, 0:256])
                        o1 = opool.tile([128, 256], bf, tag="o1")
                        nc.vector.tensor_scalar_mul(o1[:], u1b[:], r1[:])
                        nc.vector.scalar_tensor_tensor(o1[:], u2b[:], r2[:],
                                                       o1[:], mult, add)
                        # GroupNorm over the head's 256 channels
                        st6 = spool.tile([128, 6], f32, tag="st6")
                        mv = spool.tile([128, 2], f32, tag="mv")
                        nc.vector.bn_stats(st6[:], o1[:])
                        nc.vector.bn_aggr(mv[:], st6[:])
                        sd = spool.tile([128, 1], f32, tag="sd")
                        rstd = spool.tile([128, 1], f32, tag="rstd")
                        nc.vector.tensor_scalar_add(sd[:], mv[:, 1:2], GN_EPS)
                        nc.scalar.sqrt(sd[:], sd[:])
                        nc.vector.reciprocal(rstd[:], sd[:])
                        o2 = opool.tile([128, 256], bf, tag="o2")
                        nc.vector.tensor_scalar(o2[:], o1[:], mv[:, 0:1], rstd[:],
                                                sub, mult)
                        # transpose (q,ch)->(ch,q) into resident osbT
                        for half in range(2):
                            pst = pmm.tile([128, 512], bf, tag="mm", name="pst")
                            nc.tensor.transpose(pst[:, 0:128],
                                                o2[:, half * 128:(half + 1) * 128],
                                                ident[:])
                            nc.vector.tensor_copy(
                                osbT[:, 2 * h + half, b * T + jg * 128:b * T + (jg + 1) * 128],
                                pst[:, 0:128])

        # ---- phase 3: partial output projection -------------------------
        for tb in range(NTB):
            pso = [pmm.tile([128, 512], f32, tag="mm", name=f"pso{n}")
                   for n in range(4)]
            for cb in range(4):
                for n in range(4):
                    nc.tensor.matmul(pso[n][:], osbT[:, cb, tb * 128:(tb + 1) * 128],
                                     wo_sb[:, cb, n * 512:(n + 1) * 512],
                                     start=(cb == 0), stop=(cb == 3),
                                     skip_group_check=True)
            for n in range(4):
                so = stage.tile([128, 512], bf, tag="so")
                nc.scalar.copy(so[:], pso[n][:])
                nc.sync.dma_start(out_d[tb * 128:(tb + 1) * 128,
                                        n * 512:(n + 1) * 512], so[:])


# ------------------------- host side ------------------------------------

_ROPE_PERM = np.concatenate([np.arange(0, DH, 2), np.arange(1, DH, 2)])


def _prep(x, Wq, Wk, Wv, Wo, lambda_q1, lambda_k1, lambda_q2, lambda_k2,
          gn_weight, gn_bias, pos):
    lam = float(np.exp(np.sum(lambda_q1 * lambda_k1))
                - np.exp(np.sum(lambda_q2 * lambda_k2)) + LAMBDA_INIT)
    scale = DH ** -0.5

    posf = pos.astype(np.float64)
    inv = 1.0 / (ROPE_BASE ** (np.arange(0, DH, 2, dtype=np.float32) / DH))
    freqs = (posf[:, None] * inv[None, :].astype(np.float64)).astype(np.float32)
    cosv = np.cos(freqs).T          # (64, T)
    sinv = np.sin(freqs).T
    tbl = np.concatenate([np.tile(cosv, (1, B)), np.tile(sinv, (1, B))],
                         axis=0).astype(bf16)          # (128, TOK) [cos;sin]
    tbl = np.ascontiguousarray(tbl)
    tbl2 = np.concatenate([np.tile(sinv, (1, B)), np.tile(cosv, (1, B))],
                          axis=0).astype(bf16)         # (128, TOK) [sin;cos]
    tbl2 = np.ascontiguousarray(tbl2)

    # x transposed: (D, B*T) -> (TSB, 128, KB, 512) ts-major contiguous
    xT = x.reshape(TOK, D).T.astype(np.float32)
    x3 = xT.reshape(KB, 128, TSB, 512).transpose(2, 1, 0, 3)
    xT3 = np.ascontiguousarray(x3).astype(bf16)

    # Wq: (H,2,DH,D), rope-permute DH, fold score scale
    Wq4 = (Wq.reshape(H, 2, DH, D)[:, :, _ROPE_PERM, :] * scale).astype(np.float32)
    Wk4 = Wk.reshape(KV, 2, DH, D)[:, :, _ROPE_PERM, :].astype(np.float32)
    Wv3 = Wv.reshape(KV, 2 * DH, D).astype(np.float32)

    s1 = 1.0 - LAMBDA_INIT
    Wo_f = (Wo * (gn_weight * s1)[None, :]).astype(np.float32)   # (D, 4096)
    bias_out = (gn_bias * s1).astype(np.float32) @ Wo.T.astype(np.float32)

    def to_sb(w2d, cols):           # (D, cols) -> (128, KB, cols) bf16
        return np.ascontiguousarray(
            w2d.reshape(KB, 128, cols).transpose(1, 0, 2)).astype(bf16)

    in_maps = []
    for c in range(NCORES):
        wqT = Wq4[2 * c:2 * c + 2].reshape(512, D).T          # (D, 512)
        wkT = Wk4[c // 2].reshape(256, D).T                   # (D, 256)
        wvT = Wv3[c // 2].T                                   # (D, 256)
        woT = Wo_f[:, 512 * c:512 * c + 512].T                # (512 ch, D out)
        wo3 = np.ascontiguousarray(
            woT.reshape(4, 128, D).transpose(1, 0, 2)).astype(bf16)
        wq_stack = np.stack([to_sb(wqT[:, cb * 128:(cb + 1) * 128], 128)
                             for cb in range(4)])             # (4,128,KB,128)
        wk_stack = np.stack([to_sb(wkT[:, cb * 128:(cb + 1) * 128], 128)
                             for cb in range(2)])             # (2,128,KB,128)
        in_maps.append({
            "xT": xT3,
            "wqT": wq_stack,
            "wkT": wk_stack,
            "wvT": to_sb(wvT, 256),
            "woT": wo3,
            "tbl": tbl,
            "tbl2": tbl2,
        })
    return lam, in_maps, bias_out


LAST_RESULT = None


def kernel(**inputs):
    global LAST_RESULT
    inputs = {k: np.asarray(v) for k, v in inputs.items()}
    lam, in_maps, bias_out = _prep(**inputs)
    nc = build_program(lam)
    res = run_bass_kernel_spmd(nc, in_maps, core_ids=list(range(NCORES)))
    LAST_RESULT = res
    out = np.zeros((TOK, D), np.float32)
    for c in range(NCORES):
        out += res.results[c]["out"].astype(np.float32)
    out += bias_out[None, :]
    return out.reshape(B, T, D).astype(np.float32)


if __name__ == "__main__":
    import reference
    ins = {k: np.asarray(v) for k, v in reference.setup_inputs().items()}
    got = kernel(**ins)
    exp = np.asarray(reference.reference(**ins))
    rel = np.linalg.norm(got - exp) / np.linalg.norm(exp)
    print("rel err:", rel)
